# revision 21
# baseline (speedup 1.0000x reference)
"""3-layer GAT (PyG GATConv semantics) forward on 8 Trainium2 NeuronCores.

Strategy (graph/data parallel, dst-sharded):
  - Nodes padded to N_PAD = 8*98*128 and sharded by destination across 8 cores.
  - Edges (plus self-loops) bucketed host-side by (core, dst-tile, src-bank),
    sorted by dst, padded to 128-edge chunks; chunk structure equalized across
    cores so one SPMD program serves all 8.
  - Per layer: each core computes x_aug = h @ [W | W*a_src | W*a_dst] for its
    node shard (x in bf16, attention logits in f32), pushes the packed row
    table to the shared AllGather table in 4 pipelined sub-collectives, then
    processes its dst tiles: dma_gather (4 SWDGE queues) fetches x_aug rows
    by src, attention weights exp(leaky_relu(al_src + al_dst)) are computed
    per edge, and the per-dst softmax numerator/denominator are accumulated
    with one-hot matmuls on TensorE.
  - One-hot matrices come from host-precomputed bitmasks (expanded on DVE
    with bitwise_and into scaled power-of-2 one-hots in fp8; the 2^(p%8)
    scale is divided out of al_dst per edge and cancels in the softmax).
"""
import os
import numpy as np
import ml_dtypes

import concourse.bass as bass
import concourse.bacc as bacc
import concourse.tile as tile
import concourse.mybir as mybir
from concourse import ap_utils
from concourse.bass_utils import run_bass_kernel_spmd

F32 = mybir.dt.float32
BF16 = mybir.dt.bfloat16
F8 = mybir.dt.float8e4
U8 = mybir.dt.uint8
U16 = mybir.dt.uint16
I16 = mybir.dt.int16
I32 = mybir.dt.int32
AF = mybir.ActivationFunctionType
OP = mybir.AluOpType
P = 128
BF_NP = ml_dtypes.bfloat16

LAST_RESULT = {}

USE_F8 = os.environ.get("GAT_F8", "1") == "1"
AND_DIRECT = os.environ.get("GAT_AND_DIRECT", "0") == "1"
OH_DT = F8 if USE_F8 else BF16


# ----------------------------------------------------------------------------
# configuration
# ----------------------------------------------------------------------------
class Cfg:
    def __init__(self, n_nodes=100000, tiles_per_core=98,
                 ncores=8, heads=(8, 8, 1), ch=(32, 32, 40), fin0=128):
        self.n_nodes = n_nodes
        self.ncores = ncores
        self.tiles = tiles_per_core
        self.shard = tiles_per_core * P
        self.n_pad = ncores * self.shard
        assert self.n_pad >= n_nodes
        # tile groups double as gather banks (one Shared table per group):
        # group g rows are ordered (core, tile_in_group, 128), 8*25*128 <= 32767
        self.bnds = [0, 25, 50, 75, tiles_per_core]
        self.nbanks = len(self.bnds) - 1
        self.grows = [ncores * (self.bnds[g + 1] - self.bnds[g]) * P
                      for g in range(self.nbanks)]
        assert max(self.grows) <= 32768
        self.heads = list(heads)
        self.ch = list(ch)
        self.fin = [fin0, heads[0] * ch[0], heads[1] * ch[1]]
        # per-layer u16 table geometry: x cols (bf16) | al_src f32
        self.geom = []
        for l in range(3):
            xc = self.heads[l] * self.ch[l]
            elem = xc + 2 * self.heads[l]          # u16: x bf16 + al_src f32
            stride = (elem + 127) // 128 * 128
            self.geom.append(dict(xc=xc, elem=elem, stride=stride))


# ----------------------------------------------------------------------------
# host-side graph preprocessing
# ----------------------------------------------------------------------------
def _wrap_idx(flat):
    """flat[e] (e = c*128 + p) -> [128, n/16] int16 tile for dma_gather.
    HW mapping (measured): out[p, c] = table[idx_sbuf[p % 16, p//16 + 8*c]]."""
    n = len(flat)
    B = flat.reshape(n // 128, 8, 16).transpose(2, 0, 1).reshape(16, n // 16)
    return np.tile(B, (8, 1))


def preprocess(cfg, edge_index):
    # self-loops are NOT added to the gather stream: each dst tile gets a
    # "self chunk" (chunk 0) DMA-loaded straight from the core's own agin rows.
    src = np.asarray(edge_index[0]).astype(np.int64)
    dst = np.asarray(edge_index[1]).astype(np.int64)
    ne = len(src)
    core = dst // cfg.shard
    tile_ic = (dst % cfg.shard) // P
    bnds = np.asarray(cfg.bnds)
    core_s_of = src // cfg.shard
    tile_s = (src % cfg.shard) // P
    bank = np.searchsorted(bnds, tile_s, side="right") - 1
    tg = (bnds[bank + 1] - bnds[bank])
    dst_local = (dst % P).astype(np.int32)
    src_local = (core_s_of * tg * P + (tile_s - bnds[bank]) * P
                 + src % P).astype(np.int32)

    ngroups = cfg.ncores * cfg.tiles * cfg.nbanks
    key = ((core * cfg.tiles + tile_ic) * cfg.nbanks + bank).astype(np.int64)
    cnt = np.bincount(key, minlength=ngroups).reshape(cfg.ncores, cfg.tiles, cfg.nbanks)
    ch_tb = (cnt.max(axis=0) + P - 1) // P          # [tiles, nbanks] chunks, shared
    assert ch_tb.max() <= 8, f"gather call would exceed 1024 idxs: {ch_tb.max()}"
    ni_tb = ch_tb * P
    stream_len = int(ni_tb.sum())                   # per-core padded edge stream

    # static offsets of each (t, b) group in the padded stream (t-major)
    off_tb = np.zeros((cfg.tiles, cfg.nbanks), np.int64)
    acc = 0
    for t in range(cfg.tiles):
        for b in range(cfg.nbanks):
            off_tb[t, b] = acc
            acc += ni_tb[t, b]

    # scatter edges into the padded per-core streams
    order = np.argsort(key, kind="stable")
    key_s = key[order]
    group_start = np.zeros(ngroups + 1, np.int64)
    np.cumsum(np.bincount(key_s, minlength=ngroups), out=group_start[1:])
    pos_in_group = np.arange(ne, dtype=np.int64) - group_start[key_s]
    tb_flat = key_s % (cfg.tiles * cfg.nbanks)
    t_of = tb_flat // cfg.nbanks
    b_of = tb_flat % cfg.nbanks
    stream_pos = off_tb[t_of, b_of] + pos_in_group
    core_s = key_s // (cfg.tiles * cfg.nbanks)

    srcl_pad = np.zeros((cfg.ncores, stream_len), np.int32)
    dstl_pad = np.full((cfg.ncores, stream_len), 200, np.int32)  # pad sentinel
    srcl_pad[core_s, stream_pos] = src_local[order]
    dstl_pad[core_s, stream_pos] = dst_local[order]
    # the gather ucode needs >= 1 valid idx per call: force slot 0 of any
    # empty (core, t, b) group to row 0 (its dst stays the 200 sentinel)
    gcnt = cnt.astype(np.int32)                      # [ncores, tiles, nbanks]
    if gcnt.min() == 0:
        zc, zt, zb = np.nonzero(gcnt == 0)
        srcl_pad[zc, off_tb[zt, zb]] = 0
        gcnt[zc, zt, zb] = 1

    # per-tile chunk geometry (chunk 0 of every tile is the self-loop chunk)
    nchb_t = ch_tb.sum(axis=1).astype(np.int64)                 # bank chunks
    nch_t = nchb_t + 1                                          # + self chunk
    nch_max = int(nch_t.max())
    meta = dict(ch_tb=ch_tb, ni_tb=ni_tb, off_tb=off_tb, nch_t=nch_t,
                nch_max=nch_max, stream_len=stream_len)

    # per-core aux blob: per tile [128, KA] u8 = [idx | dcol bf16]; plus a
    # per-tile [1, E] drow stream (dst-local along the edge axis)
    self_dc = np.arange(P, dtype=np.float32).astype(BF_NP)
    aux_flats, dr_flats = [], []
    idx_boff = np.zeros((cfg.tiles, cfg.nbanks), np.int64)   # i16 col offset
    off_dc = np.zeros(cfg.tiles, np.int64)
    ka_t = np.zeros(cfg.tiles, np.int64)
    dr_off = np.zeros(cfg.tiles, np.int64)
    for c in range(cfg.ncores):
        tile_blobs, dr_parts = [], []
        for t in range(cfg.tiles):
            nch = int(nch_t[t])
            nchb = nch - 1
            seg0 = int(off_tb[t, 0])
            nt = nchb * P
            dl = dstl_pad[c, seg0:seg0 + nt]
            dc = np.ascontiguousarray(np.concatenate(
                [self_dc[:, None], dl.reshape(-1, P).T.astype(BF_NP)], axis=1))
            if c == 0:
                dr_off[t] = sum(x.size for x in dr_parts)
            dr_parts.append(np.concatenate([self_dc, dl.astype(BF_NP)]))
            blocks = []
            coloff = 0
            for b in range(cfg.nbanks):
                ni = int(ni_tb[t, b])
                if ni == 0:
                    continue
                if c == 0:
                    idx_boff[t, b] = coloff
                blocks.append(_wrap_idx(
                    srcl_pad[c, int(off_tb[t, b]):int(off_tb[t, b]) + ni]
                    .astype(np.int16)))
                coloff += ni // 16
            tile_idx = np.concatenate(blocks, axis=1)   # [128, nchb*8] i16
            if c == 0:
                off_dc[t] = tile_idx.shape[1] * 2
                ka_t[t] = off_dc[t] + nch * 2
            blob = np.concatenate(
                [tile_idx.view(np.uint8).reshape(P, -1),
                 dc.view(np.uint8).reshape(P, -1)], axis=1)
            tile_blobs.append(blob.ravel())
        aux_flats.append(np.concatenate(tile_blobs))
        dr_flats.append(np.concatenate(dr_parts))
    aux_off = np.zeros(cfg.tiles, np.int64)
    np.cumsum(P * ka_t[:-1], out=aux_off[1:])
    gcnt_flats = [gcnt[c].reshape(-1) for c in range(cfg.ncores)]
    meta.update(idx_boff=idx_boff, off_dc=off_dc,
                ka_t=ka_t, aux_off=aux_off, aux_len=len(aux_flats[0]),
                dr_off=dr_off, dr_len=len(dr_flats[0]))
    return meta, aux_flats, dr_flats, gcnt_flats


def make_weights(cfg, inputs):
    """Per-layer: W_x bf16 [fin, xc]; W_al f32 [fin, 2H]; b_rep f32 [128, xc]."""
    out = {}
    for l in range(3):
        W = np.asarray(inputs[f"W{l}"], np.float32)
        a_src = np.asarray(inputs[f"a_src{l}"], np.float32)
        a_dst = np.asarray(inputs[f"a_dst{l}"], np.float32)
        b = np.asarray(inputs[f"b{l}"], np.float32)
        H, C = a_src.shape
        wal = np.zeros((W.shape[0], 2 * H), np.float32)
        for h in range(H):
            wal[:, h] = W[:, h * C:(h + 1) * C] @ a_src[h]
            wal[:, H + h] = W[:, h * C:(h + 1) * C] @ a_dst[h]
        out[f"wx{l}"] = W.astype(BF_NP)
        out[f"wal{l}"] = wal
        out[f"brep{l}"] = np.broadcast_to(b, (P, len(b))).copy()
    return out


# ----------------------------------------------------------------------------
# patched dma_gather (non-transpose HBM source; elem bytes need not be %256)
# ----------------------------------------------------------------------------
def dma_gather_unaligned(gpsimd, out_ap, in_ap, idxs_ap, num_idxs, elem_size,
                         elem_step, queue_num=0, num_idxs_reg=None):
    """num_idxs is the static slot count (idx array / out shape); num_idxs_reg
    is the runtime count of valid (non-negative) idxs — the engine skips the
    trailing -1 padding, so descriptors are only generated for real edges."""
    assert idxs_ap.dtype == I16
    assert in_ap.dtype == out_ap.dtype
    assert ap_utils.ap_is_contiguous(in_ap.ap[1:])
    assert ap_utils.ap_is_contiguous(out_ap.ap[1:])
    assert ap_utils.ap_is_contiguous(idxs_ap.ap[1:])
    assert in_ap.ap[-1][1] == out_ap.ap[-1][1] == elem_size
    assert out_ap.ap[0][1] * out_ap.ap[1][1] == (num_idxs + 127) // 128 * 128
    assert in_ap.ap[0][0] == elem_step
    dtsz = mybir.dt.size(in_ap.dtype)
    stride_bytes = elem_step * dtsz
    assert stride_bytes % 256 == 0 and stride_bytes // 256 < 256
    if num_idxs_reg is None:
        num_idxs_reg = num_idxs
    _in_ap = gpsimd.lower_ap_dma(in_ap, for_custom_bir_dma=True)
    _idxs_ap = gpsimd.lower_ap(idxs_ap)
    _out_ap = gpsimd.lower_ap(out_ap)
    return gpsimd.add_instruction(
        mybir.InstDMAGatherAnt(
            name=gpsimd.bass.get_next_instruction_name(),
            ins=[*_in_ap, _idxs_ap,
                 gpsimd.lower_val_access(gpsimd.to_reg(num_idxs_reg))],
            outs=[_out_ap],
            transpose=False, num_idxs=num_idxs, elem_size=elem_size,
            stride_bytes_256=stride_bytes // 256, gen_mode=0,
            single_packet=True, queue_num=queue_num,
            sbuf_tokens_per_rank=0, sbuf_free_dim_per_rank=0,
            sbuf_free_dim_pad_per_rank=0, sbuf_byte_offset=0,
        ))


# ----------------------------------------------------------------------------
# kernel builder
# ----------------------------------------------------------------------------
def build(cfg, meta):
    nc = bacc.Bacc("TRN2", target_bir_lowering=False, debug=False,
                   num_devices=cfg.ncores, num_swdge_queues=4,
                   dynamic_dma_scratch_size=32768)

    feats = nc.dram_tensor("feats", [cfg.shard, cfg.fin[0]], F32, kind="ExternalInput")
    aux = nc.dram_tensor("aux", [meta["aux_len"]], U8, kind="ExternalInput")
    wx, wal, brep = [], [], []
    for l in range(3):
        wx.append(nc.dram_tensor(f"wx{l}", [cfg.fin[l], cfg.geom[l]["xc"]], BF16,
                                 kind="ExternalInput"))
        wal.append(nc.dram_tensor(f"wal{l}", [cfg.fin[l], 2 * cfg.heads[l]], F32,
                                  kind="ExternalInput"))
        brep.append(nc.dram_tensor(f"brep{l}", [P, cfg.geom[l]["xc"]], F32,
                                   kind="ExternalInput"))
    ident_in = nc.dram_tensor("ident", [P, P], F32, kind="ExternalInput")
    iota_in = nc.dram_tensor("iota", [P, P], BF16, kind="ExternalInput")
    drow = nc.dram_tensor("drow", [meta["dr_len"]], BF16, kind="ExternalInput")
    iotacf_in = nc.dram_tensor("iotacf", [P, 1], F32, kind="ExternalInput")
    ones_in = nc.dram_tensor("ones", [1, P], BF16, kind="ExternalInput")
    out_sh = nc.dram_tensor("out_shard", [cfg.shard, cfg.ch[2]], F32,
                            kind="ExternalOutput")

    NQ = int(os.environ.get("GAT_NQ", "4"))
    ch_tb, ni_tb, nch_t = meta["ch_tb"], meta["ni_tb"], meta["nch_t"]
    idx_boff = meta["idx_boff"]
    off_dc, ka_t = meta["off_dc"], meta["ka_t"]
    aux_off = meta["aux_off"]
    NCH = meta["nch_max"]
    KA = int(ka_t.max())
    # sub-AllGather tile-group boundaries (pipelined collective)
    NSUB = cfg.nbanks
    bnds = cfg.bnds

    with tile.TileContext(nc) as tc:
        with (
            tc.tile_pool(name="const", bufs=1) as cp,
            tc.tile_pool(name="sb", bufs=2) as sp,
            tc.tile_pool(name="ps", bufs=1, space="PSUM") as pp,
            tc.tile_pool(name="dram", bufs=1, space="DRAM") as dp,
        ):
            # ---------------- constants ----------------
            ident = cp.tile([P, P], F32)
            nc.sync.dma_start(out=ident[:], in_=ident_in[:, :])
            iota = cp.tile([P, P], BF16)
            nc.sync.dma_start(out=iota[:], in_=iota_in[:, :])
            iotacf = cp.tile([P, 1], F32)
            nc.sync.dma_start(out=iotacf[:], in_=iotacf_in[:, :])
            ones = cp.tile([1, P], BF16)
            nc.sync.dma_start(out=ones[:], in_=ones_in[:, :])
            wx_t, wal_t, b_t, hilo_all = [], [], [], []
            for l in range(3):
                nf = cfg.fin[l] // P
                t = cp.tile([P, nf, cfg.geom[l]["xc"]], BF16, name=f"wxt{l}")
                nc.sync.dma_start(
                    out=t[:], in_=wx[l].ap().rearrange("(f p) c -> p f c", p=P))
                wx_t.append(t)
                t = cp.tile([P, nf, 2 * cfg.heads[l]], F32, name=f"walt{l}")
                nc.sync.dma_start(
                    out=t[:], in_=wal[l].ap().rearrange("(f p) c -> p f c", p=P))
                wal_t.append(t)
                t = cp.tile([P, cfg.geom[l]["xc"]], F32, name=f"bt{l}")
                nc.sync.dma_start(out=t[:], in_=brep[l].ap())
                b_t.append(t)
                hilo_all.append(cp.tile([P, cfg.tiles, 2 * cfg.heads[l]], BF16,
                                        name=f"hilo{l}"))

            # ---------------- DRAM bounces ----------------
            agin, table = [], []
            for l in range(3):
                st = cfg.geom[l]["stride"]
                agin.append(dp.tile([cfg.shard, st], U16, name=f"agin{l}"))
                tl = []
                for g in range(cfg.nbanks):
                    tl.append(dp.tile([cfg.grows[g], st], U16,
                                      name=f"table{l}_{g}", addr_space="Shared"))
                table.append(tl)

            rg = [list(range(cfg.ncores))]

            def launch_sub(l, k):
                a, b = bnds[k], bnds[k + 1]
                nc.gpsimd.collective_compute(
                    "AllGather", OP.bypass,
                    ins=[agin[l][a * P:b * P, :].opt()],
                    outs=[table[l][k][:].opt()],
                    replica_groups=rg)

            # ---------------- helpers ----------------
            def phase_a(l, t, h_tile):
                """h_tile: [128, fin] f32 SBUF -> writes agin[l] rows of tile t
                and this core's al_dst hi/lo pair into hilo_all[l]."""
                g = cfg.geom[l]
                H_ = cfg.heads[l]
                xc = g["xc"]
                nf = cfg.fin[l] // P
                hT = sp.tile([P, nf, P], F32, tag="hT")
                hTb = sp.tile([P, nf, P], BF16, tag="hTb")
                for f in range(nf):
                    tp = pp.tile([P, P], F32, space="PSUM", tag="scr", bufs=2)
                    nc.tensor.transpose(out=tp[:], in_=h_tile[:, f * P:(f + 1) * P],
                                        identity=ident[:])
                    nc.vector.tensor_copy(out=hT[:, f, :], in_=tp[:])
                    nc.scalar.activation(out=hTb[:, f, :], in_=hT[:, f, :],
                                         func=AF.Copy)
                aps = pp.tile([P, xc + 2 * H_], F32, space="PSUM",
                              tag="aps", bufs=2)
                for f in range(nf):
                    nc.tensor.matmul(out=aps[:, 0:xc], lhsT=hTb[:, f, :],
                                     rhs=wx_t[l][:, f, :],
                                     start=(f == 0), stop=(f == nf - 1))
                for f in range(nf):
                    nc.tensor.matmul(out=aps[:, xc:], lhsT=hT[:, f, :],
                                     rhs=wal_t[l][:, f, :],
                                     start=(f == 0), stop=(f == nf - 1))
                row = sp.tile([P, g["stride"]], U16, tag="row")
                rb = row[:].bitcast(BF16)
                nc.scalar.activation(out=rb[:, 0:xc], in_=aps[:, 0:xc],
                                     func=AF.Copy)
                rf = row[:].bitcast(F32)
                nc.vector.tensor_copy(out=rf[:, xc // 2:xc // 2 + H_],
                                      in_=aps[:, xc:xc + H_])
                # al_dst as bf16 hi/lo pair -> SBUF-resident hilo_all
                hi_sl = hilo_all[l][:, t, 0:H_]
                nc.scalar.activation(out=hi_sl, in_=aps[:, xc + H_:], func=AF.Copy)
                nc.vector.tensor_tensor(out=hilo_all[l][:, t, H_:2 * H_],
                                        in0=aps[:, xc + H_:], in1=hi_sl,
                                        op=OP.subtract)
                nc.scalar.dma_start(out=agin[l][t * P:(t + 1) * P, :], in_=row[:])

            def edge_loads(l, t):
                """stage 0: gt self chunk + gathers + one aux DMA."""
                g = cfg.geom[l]
                nch = int(nch_t[t])
                nchb = nch - 1

                gt_full = sp.tile([P, NCH, g["elem"]], U16, tag="g", bufs=3,
                                  name="gt")
                gt = gt_full[:, 0:nch, :]
                if t < 3:
                    # first rotation of each layer: zero the whole buffer so
                    # slots skipped by the trimmed gather hold finite bits at
                    # THIS layer's elem alignment (misaligned stale data can
                    # reinterpret as f32 NaN/huge -> exp -> Inf*0 = NaN)
                    nc.vector.memset(gt_full[:], 0)
                # chunk 0 = self-loop rows: this core's own x_aug tile
                nc.sync.dma_start(out=gt[:, 0, :],
                                  in_=agin[l][t * P:(t + 1) * P, 0:g["elem"]])
                axt = sp.tile([P, KA], U8, tag="aux", bufs=4)
                ka = int(ka_t[t])
                nc.sync.dma_start(
                    out=axt[:, 0:ka],
                    in_=aux.ap()[int(aux_off[t]):int(aux_off[t]) + P * ka]
                    .rearrange("(p m) -> p m", p=P))
                ax16 = axt[:].bitcast(I16)
                coff = 1
                for b in range(cfg.nbanks):
                    chb = int(ch_tb[t, b])
                    if chb == 0:
                        continue
                    ni = chb * P
                    rows = cfg.grows[b]
                    dma_gather_unaligned(
                        nc.gpsimd,
                        out_ap=gt[:, coff:coff + chb, :],
                        in_ap=table[l][b][0:rows, 0:g["elem"]],
                        idxs_ap=ax16[:, int(idx_boff[t, b]):
                                     int(idx_boff[t, b]) + ni // 16],
                        num_idxs=ni, elem_size=g["elem"],
                        elem_step=g["stride"], queue_num=(t + b) % NQ)
                    coff += chb
                dcol_t = axt[:].bitcast(BF16)[:, int(off_dc[t]) // 2:
                                              int(off_dc[t]) // 2 + nch]
                E = nch * P
                drow_t = sp.tile([1, NCH * P], BF16, tag="drow", bufs=3,
                                 name="drow_t")[:, 0:E]
                nc.sync.dma_start(
                    out=drow_t,
                    in_=drow.ap()[int(meta["dr_off"][t]):
                                  int(meta["dr_off"][t]) + E].unsqueeze(0))
                return dict(gt=gt, dcol_t=dcol_t, drow_t=drow_t)

            def edge_front(l, t, ld):
                """one-hot builds + al_dst expansion (deps: loads of t only)."""
                H = cfg.heads[l]
                nch = int(nch_t[t])
                E = nch * P
                dcol_t, drow_t = ld["dcol_t"], ld["drow_t"]

                oh = sp.tile([P, NCH, P], OH_DT, tag="oh", bufs=3, name="oh")[:, 0:nch, :]
                nc.vector.tensor_tensor(
                    out=oh,
                    in0=dcol_t.unsqueeze(2).to_broadcast([P, nch, P]),
                    in1=iota[:].unsqueeze(1).to_broadcast([P, nch, P]),
                    op=OP.is_equal)
                # broadcast drow across partitions via ones-matmul, then
                # ohTm[j, e] = 1 iff dst(e) == j
                dstb = sp.tile([P, NCH * P], BF16, tag="dstb", bufs=3,
                               name="dstb")[:, 0:E]
                for s0 in range(0, E, 512):
                    s1 = min(s0 + 512, E)
                    bc = pp.tile([P, 512], F32, space="PSUM", tag="scr", bufs=2,
                                 name="bc")
                    nc.tensor.matmul(out=bc[:, 0:s1 - s0], lhsT=ones[:],
                                     rhs=drow_t[:, s0:s1], start=True, stop=True)
                    nc.scalar.activation(out=dstb[:, s0:s1],
                                         in_=bc[:, 0:s1 - s0], func=AF.Copy)
                ohTm = sp.tile([P, NCH, P], OH_DT, tag="ohT", bufs=3,
                               name="ohTm")[:, 0:nch, :]
                nc.vector.tensor_scalar(
                    out=ohTm.rearrange("p c k -> p (c k)"), in0=dstb,
                    scalar1=iotacf[:, 0:1], scalar2=0.0,
                    op0=OP.subtract, op1=OP.is_equal)
                adx = pp.tile([P, NCH * H], F32, space="PSUM", tag="adx",
                              bufs=2, name="adx")[:, 0:nch * H]
                hi = hilo_all[l][:, t, 0:H]
                lo = hilo_all[l][:, t, H:2 * H]
                for c in range(nch):
                    nc.tensor.matmul(out=adx[:, c * H:(c + 1) * H],
                                     lhsT=ohTm[:, c, :],
                                     rhs=hi, start=True, stop=False)
                    nc.tensor.matmul(out=adx[:, c * H:(c + 1) * H],
                                     lhsT=ohTm[:, c, :],
                                     rhs=lo, start=False, stop=True)
                return dict(oh=oh, adx=adx)

            def edge_back(l, t, ld, fr):
                """attention weights + weighted values + segment sums."""
                g = cfg.geom[l]
                H = cfg.heads[l]
                C = cfg.ch[l]
                xc = g["xc"]
                nch = int(nch_t[t])
                gt, oh, adx = ld["gt"], fr["oh"], fr["adx"]

                gf = gt[:].bitcast(F32)
                alsrc = gf[:, :, xc // 2:xc // 2 + H]
                S = sp.tile([P, NCH, H], F32, tag="S", bufs=3, name="S")[:, 0:nch, :]
                nc.vector.tensor_tensor(
                    out=S, in0=alsrc, in1=adx.rearrange("p (c k) -> p c k", k=H),
                    op=OP.add)
                S2 = sp.tile([P, NCH, H], F32, tag="S2", bufs=3, name="S2")[:, 0:nch, :]
                nc.vector.scalar_tensor_tensor(out=S2, in0=S, scalar=0.2,
                                               in1=S, op0=OP.mult, op1=OP.max)
                gb = gt[:].bitcast(BF16)
                v = sp.tile([P, NCH, xc + H], BF16, tag="v", bufs=2, name="v")[:, 0:nch, :]
                ew = v[:, :, xc:xc + H]
                nc.scalar.activation(out=ew, in_=S2, func=AF.Exp)
                wexp = sp.tile([P, NCH, xc], BF16, tag="wexp", bufs=2,
                               name="wexp")[:, 0:nch, :]
                nc.scalar.activation(
                    out=wexp.rearrange("p c (h x) -> p c h x", h=H),
                    in_=S2.unsqueeze(3).to_broadcast([P, nch, H, C]),
                    func=AF.Exp)
                nc.vector.tensor_tensor(out=v[:, :, 0:xc], in0=gb[:, :, 0:xc],
                                        in1=wexp, op=OP.mult)
                ops = pp.tile([P, xc + H], F32, space="PSUM", tag="ops", bufs=2)
                for c in range(nch):
                    nc.tensor.matmul(out=ops[:], lhsT=oh[:, c, :],
                                     rhs=v[:, c, :],
                                     start=(c == 0), stop=(c == nch - 1))
                return ops

            def edge_epi(l, t, ops):
                """normalize + bias (+ ELU); returns h_next or writes out."""
                g = cfg.geom[l]
                H = cfg.heads[l]
                C = cfg.ch[l]
                xc = g["xc"]
                se = sp.tile([P, H], F32, tag="se", bufs=3)
                nc.vector.tensor_scalar_add(out=se[:], in0=ops[:, xc:xc + H],
                                            scalar1=1e-30)
                rs = sp.tile([P, H], F32, tag="rs", bufs=3)
                nc.vector.reciprocal(out=rs[:], in_=se[:])
                h1 = sp.tile([P, xc], F32, tag="h1", bufs=3)
                nc.vector.tensor_tensor(
                    out=h1[:].rearrange("p (h x) -> p h x", h=H),
                    in0=ops[:, 0:xc].rearrange("p (h x) -> p h x", h=H),
                    in1=rs[:].unsqueeze(2).to_broadcast([P, H, C]),
                    op=OP.mult)
                h2 = sp.tile([P, xc], F32, tag="h2", bufs=3)
                nc.vector.tensor_tensor(out=h2[:], in0=h1[:], in1=b_t[l][:],
                                        op=OP.add)
                if l == 2:
                    nc.scalar.dma_start(out=out_sh[t * P:(t + 1) * P, :], in_=h2[:])
                    return None
                m = sp.tile([P, xc], F32, tag="m", bufs=3)
                nc.vector.tensor_scalar_min(out=m[:], in0=h2[:], scalar1=0.0)
                nc.scalar.activation(out=m[:], in_=m[:], func=AF.Exp)
                hn = sp.tile([P, xc], F32, tag="hn", bufs=3)
                nc.vector.scalar_tensor_tensor(out=hn[:], in0=m[:], scalar=-1.0,
                                               in1=h2[:], op0=OP.add, op1=OP.max)
                return hn

            # ---------------- program ----------------
            sub_at = {bnds[k + 1] - 1: k for k in range(NSUB)}

            def edge_phase(l, next_l):
                lds, frs, opss = {}, {}, {}
                T = cfg.tiles
                for t in range(T + 3):
                    if t < T:
                        lds[t] = edge_loads(l, t)
                    if t - 1 >= 0 and t - 1 < T:
                        frs[t - 1] = edge_front(l, t - 1, lds[t - 1])
                    if t - 2 >= 0 and t - 2 < T:
                        u = t - 2
                        opss[u] = edge_back(l, u, lds.pop(u), frs.pop(u))
                    if t - 3 >= 0 and t - 3 < T:
                        u = t - 3
                        hn = edge_epi(l, u, opss.pop(u))
                        if next_l is not None:
                            phase_a(next_l, u, hn)
                            if u in sub_at:
                                launch_sub(next_l, sub_at[u])

            # layer 0 phase A from features (sub-collectives interleaved)
            for t in range(cfg.tiles):
                h0 = sp.tile([P, cfg.fin[0]], F32, tag="h0")
                nc.sync.dma_start(out=h0[:], in_=feats.ap()[t * P:(t + 1) * P, :])
                phase_a(0, t, h0)
                if t in sub_at:
                    launch_sub(0, sub_at[t])
            edge_phase(0, 1)
            edge_phase(1, 2)
            edge_phase(2, None)

    nc.compile()
    return nc


# ----------------------------------------------------------------------------
# entry point
# ----------------------------------------------------------------------------
def run_gat(cfg, inputs, trace=False):
    meta, aux_flats, dr_flats, gcnt_flats = preprocess(cfg, inputs["edge_index"])
    wts = make_weights(cfg, inputs)
    feats = np.asarray(inputs["features"], np.float32)
    feats_pad = np.zeros((cfg.n_pad, cfg.fin[0]), np.float32)
    feats_pad[:cfg.n_nodes] = feats

    nc = build(cfg, meta)

    shared = dict(wts)
    shared["ident"] = np.eye(P, dtype=np.float32)
    shared["iota"] = np.broadcast_to(np.arange(P, dtype=np.float32), (P, P)).astype(BF_NP)
    shared["iotacf"] = np.arange(P, dtype=np.float32).reshape(P, 1)
    shared["ones"] = np.ones((1, P), BF_NP)
    in_maps = []
    for c in range(cfg.ncores):
        m = dict(shared)
        m["feats"] = feats_pad[c * cfg.shard:(c + 1) * cfg.shard]
        m["aux"] = aux_flats[c]
        m["drow"] = dr_flats[c]
        in_maps.append(m)

    res = run_bass_kernel_spmd(nc, in_maps, core_ids=list(range(cfg.ncores)),
                               trace=trace)
    LAST_RESULT["exec_time_ns"] = res.exec_time_ns
    out = np.concatenate([res.results[c]["out_shard"] for c in range(cfg.ncores)],
                         axis=0)[:cfg.n_nodes]
    return out


def kernel(**inputs):
    cfg = Cfg()
    trace = os.environ.get("GAT_TRACE", "0") == "1"
    if trace:
        try:
            import sys as _sys, types as _types
            import trn_agent_boot.trn_boot as _tb
            _m = _types.ModuleType("antenv.axon_hooks")
            _hook = _tb._ntff_profile_via_ctypes("/opt/axon/libaxon_pjrt.so")
            _m.get_axon_ntff_profile_hook = lambda: _hook
            _m.set_axon_ntff_profile_hook = lambda h: None
            _sys.modules.setdefault("antenv.axon_hooks", _m)
            import concourse.bass_utils as _bu
            _bu.upload_artifacts = lambda tmpdir: f"file://{tmpdir}"
        except Exception:
            trace = False
    return run_gat(cfg, inputs, trace=trace).astype(np.float32)


# revision 23
# speedup vs baseline: 1.0178x; 1.0178x over previous
"""3-layer GAT (PyG GATConv semantics) forward on 8 Trainium2 NeuronCores.

Strategy (graph/data parallel, dst-sharded):
  - Nodes padded to N_PAD = 8*98*128 and sharded by destination across 8 cores.
  - Edges (plus self-loops) bucketed host-side by (core, dst-tile, src-bank),
    sorted by dst, padded to 128-edge chunks; chunk structure equalized across
    cores so one SPMD program serves all 8.
  - Per layer: each core computes x_aug = h @ [W | W*a_src | W*a_dst] for its
    node shard (x in bf16, attention logits in f32), pushes the packed row
    table to the shared AllGather table in 4 pipelined sub-collectives, then
    processes its dst tiles: dma_gather (4 SWDGE queues) fetches x_aug rows
    by src, attention weights exp(leaky_relu(al_src + al_dst)) are computed
    per edge, and the per-dst softmax numerator/denominator are accumulated
    with one-hot matmuls on TensorE.
  - One-hot matrices come from host-precomputed bitmasks (expanded on DVE
    with bitwise_and into scaled power-of-2 one-hots in fp8; the 2^(p%8)
    scale is divided out of al_dst per edge and cancels in the softmax).
"""
import os
import numpy as np
import ml_dtypes

import concourse.bass as bass
import concourse.bacc as bacc
import concourse.tile as tile
import concourse.mybir as mybir
from concourse import ap_utils
from concourse.bass_utils import run_bass_kernel_spmd

F32 = mybir.dt.float32
BF16 = mybir.dt.bfloat16
F8 = mybir.dt.float8e4
U8 = mybir.dt.uint8
U16 = mybir.dt.uint16
I16 = mybir.dt.int16
I32 = mybir.dt.int32
AF = mybir.ActivationFunctionType
OP = mybir.AluOpType
P = 128
BF_NP = ml_dtypes.bfloat16

LAST_RESULT = {}

USE_F8 = os.environ.get("GAT_F8", "1") == "1"
AND_DIRECT = os.environ.get("GAT_AND_DIRECT", "0") == "1"
OH_DT = F8 if USE_F8 else BF16


# ----------------------------------------------------------------------------
# configuration
# ----------------------------------------------------------------------------
class Cfg:
    def __init__(self, n_nodes=100000, tiles_per_core=98,
                 ncores=8, heads=(8, 8, 1), ch=(32, 32, 40), fin0=128):
        self.n_nodes = n_nodes
        self.ncores = ncores
        self.tiles = tiles_per_core
        self.shard = tiles_per_core * P
        self.n_pad = ncores * self.shard
        assert self.n_pad >= n_nodes
        # tile groups double as gather banks (one Shared table per group):
        # group g rows are ordered (core, tile_in_group, 128), 8*25*128 <= 32767
        self.bnds = [0, 25, 50, 75, tiles_per_core]
        self.nbanks = len(self.bnds) - 1
        self.grows = [ncores * (self.bnds[g + 1] - self.bnds[g]) * P
                      for g in range(self.nbanks)]
        assert max(self.grows) <= 32768
        self.heads = list(heads)
        self.ch = list(ch)
        self.fin = [fin0, heads[0] * ch[0], heads[1] * ch[1]]
        # per-layer u16 table geometry: x cols (bf16) | al_src f32
        self.geom = []
        for l in range(3):
            xc = self.heads[l] * self.ch[l]
            elem = xc + 2 * self.heads[l]          # u16: x bf16 + al_src f32
            stride = (elem + 127) // 128 * 128
            self.geom.append(dict(xc=xc, elem=elem, stride=stride))


# ----------------------------------------------------------------------------
# host-side graph preprocessing
# ----------------------------------------------------------------------------
def _wrap_idx(flat):
    """flat[e] (e = c*128 + p) -> [128, n/16] int16 tile for dma_gather.
    HW mapping (measured): out[p, c] = table[idx_sbuf[p % 16, p//16 + 8*c]]."""
    n = len(flat)
    B = flat.reshape(n // 128, 8, 16).transpose(2, 0, 1).reshape(16, n // 16)
    return np.tile(B, (8, 1))


def preprocess(cfg, edge_index):
    # self-loops are NOT added to the gather stream: each dst tile gets a
    # "self chunk" (chunk 0) DMA-loaded straight from the core's own agin rows.
    src = np.asarray(edge_index[0]).astype(np.int64)
    dst = np.asarray(edge_index[1]).astype(np.int64)
    ne = len(src)
    core = dst // cfg.shard
    tile_ic = (dst % cfg.shard) // P
    bnds = np.asarray(cfg.bnds)
    core_s_of = src // cfg.shard
    tile_s = (src % cfg.shard) // P
    bank = np.searchsorted(bnds, tile_s, side="right") - 1
    tg = (bnds[bank + 1] - bnds[bank])
    dst_local = (dst % P).astype(np.int32)
    src_local = (core_s_of * tg * P + (tile_s - bnds[bank]) * P
                 + src % P).astype(np.int32)

    ngroups = cfg.ncores * cfg.tiles * cfg.nbanks
    key = ((core * cfg.tiles + tile_ic) * cfg.nbanks + bank).astype(np.int64)
    cnt = np.bincount(key, minlength=ngroups).reshape(cfg.ncores, cfg.tiles, cfg.nbanks)
    ch_tb = (cnt.max(axis=0) + P - 1) // P          # [tiles, nbanks] chunks, shared
    assert ch_tb.max() <= 8, f"gather call would exceed 1024 idxs: {ch_tb.max()}"
    ni_tb = ch_tb * P
    stream_len = int(ni_tb.sum())                   # per-core padded edge stream

    # static offsets of each (t, b) group in the padded stream (t-major)
    off_tb = np.zeros((cfg.tiles, cfg.nbanks), np.int64)
    acc = 0
    for t in range(cfg.tiles):
        for b in range(cfg.nbanks):
            off_tb[t, b] = acc
            acc += ni_tb[t, b]

    # scatter edges into the padded per-core streams
    order = np.argsort(key, kind="stable")
    key_s = key[order]
    group_start = np.zeros(ngroups + 1, np.int64)
    np.cumsum(np.bincount(key_s, minlength=ngroups), out=group_start[1:])
    pos_in_group = np.arange(ne, dtype=np.int64) - group_start[key_s]
    tb_flat = key_s % (cfg.tiles * cfg.nbanks)
    t_of = tb_flat // cfg.nbanks
    b_of = tb_flat % cfg.nbanks
    stream_pos = off_tb[t_of, b_of] + pos_in_group
    core_s = key_s // (cfg.tiles * cfg.nbanks)

    srcl_pad = np.zeros((cfg.ncores, stream_len), np.int32)
    dstl_pad = np.full((cfg.ncores, stream_len), 200, np.int32)  # pad sentinel
    srcl_pad[core_s, stream_pos] = src_local[order]
    dstl_pad[core_s, stream_pos] = dst_local[order]
    # the gather ucode needs >= 1 valid idx per call: force slot 0 of any
    # empty (core, t, b) group to row 0 (its dst stays the 200 sentinel)
    gcnt = cnt.astype(np.int32)                      # [ncores, tiles, nbanks]
    if gcnt.min() == 0:
        zc, zt, zb = np.nonzero(gcnt == 0)
        srcl_pad[zc, off_tb[zt, zb]] = 0
        gcnt[zc, zt, zb] = 1

    # per-tile chunk geometry (chunk 0 of every tile is the self-loop chunk)
    nchb_t = ch_tb.sum(axis=1).astype(np.int64)                 # bank chunks
    nch_t = nchb_t + 1                                          # + self chunk
    nch_max = int(nch_t.max())
    meta = dict(ch_tb=ch_tb, ni_tb=ni_tb, off_tb=off_tb, nch_t=nch_t,
                nch_max=nch_max, stream_len=stream_len)

    # per-core aux blob: per tile [128, KA] u8 = [idx | dcol bf16]; plus a
    # per-tile [1, E] drow stream (dst-local along the edge axis)
    self_dc = np.arange(P, dtype=np.float32).astype(BF_NP)
    aux_flats, dr_flats = [], []
    idx_boff = np.zeros((cfg.tiles, cfg.nbanks), np.int64)   # i16 col offset
    off_dc = np.zeros(cfg.tiles, np.int64)
    ka_t = np.zeros(cfg.tiles, np.int64)
    dr_off = np.zeros(cfg.tiles, np.int64)
    for c in range(cfg.ncores):
        tile_blobs, dr_parts = [], []
        for t in range(cfg.tiles):
            nch = int(nch_t[t])
            nchb = nch - 1
            seg0 = int(off_tb[t, 0])
            nt = nchb * P
            dl = dstl_pad[c, seg0:seg0 + nt]
            dc = np.ascontiguousarray(np.concatenate(
                [self_dc[:, None], dl.reshape(-1, P).T.astype(BF_NP)], axis=1))
            if c == 0:
                dr_off[t] = sum(x.size for x in dr_parts)
            dr_parts.append(np.concatenate([self_dc, dl.astype(BF_NP)]))
            blocks = []
            coloff = 0
            for b in range(cfg.nbanks):
                ni = int(ni_tb[t, b])
                if ni == 0:
                    continue
                if c == 0:
                    idx_boff[t, b] = coloff
                blocks.append(_wrap_idx(
                    srcl_pad[c, int(off_tb[t, b]):int(off_tb[t, b]) + ni]
                    .astype(np.int16)))
                coloff += ni // 16
            tile_idx = np.concatenate(blocks, axis=1)   # [128, nchb*8] i16
            if c == 0:
                off_dc[t] = tile_idx.shape[1] * 2
                ka_t[t] = off_dc[t] + nch * 2
            blob = np.concatenate(
                [tile_idx.view(np.uint8).reshape(P, -1),
                 dc.view(np.uint8).reshape(P, -1)], axis=1)
            tile_blobs.append(blob.ravel())
        aux_flats.append(np.concatenate(tile_blobs))
        dr_flats.append(np.concatenate(dr_parts))
    aux_off = np.zeros(cfg.tiles, np.int64)
    np.cumsum(P * ka_t[:-1], out=aux_off[1:])
    gcnt_flats = [gcnt[c].reshape(-1) for c in range(cfg.ncores)]
    meta.update(idx_boff=idx_boff, off_dc=off_dc,
                ka_t=ka_t, aux_off=aux_off, aux_len=len(aux_flats[0]),
                dr_off=dr_off, dr_len=len(dr_flats[0]))
    return meta, aux_flats, dr_flats, gcnt_flats


def make_weights(cfg, inputs):
    """Per-layer: W_x bf16 [fin, xc]; W_al f32 [fin, 2H]; b_rep f32 [128, xc]."""
    out = {}
    for l in range(3):
        W = np.asarray(inputs[f"W{l}"], np.float32)
        a_src = np.asarray(inputs[f"a_src{l}"], np.float32)
        a_dst = np.asarray(inputs[f"a_dst{l}"], np.float32)
        b = np.asarray(inputs[f"b{l}"], np.float32)
        H, C = a_src.shape
        wal = np.zeros((W.shape[0], 2 * H), np.float32)
        for h in range(H):
            wal[:, h] = W[:, h * C:(h + 1) * C] @ a_src[h]
            wal[:, H + h] = W[:, h * C:(h + 1) * C] @ a_dst[h]
        out[f"wx{l}"] = W.astype(BF_NP)
        out[f"wal{l}"] = wal
        out[f"brep{l}"] = np.broadcast_to(b, (P, len(b))).copy()
    return out


# ----------------------------------------------------------------------------
# patched dma_gather (non-transpose HBM source; elem bytes need not be %256)
# ----------------------------------------------------------------------------
def dma_gather_unaligned(gpsimd, out_ap, in_ap, idxs_ap, num_idxs, elem_size,
                         elem_step, queue_num=0, num_idxs_reg=None):
    """num_idxs is the static slot count (idx array / out shape); num_idxs_reg
    is the runtime count of valid (non-negative) idxs — the engine skips the
    trailing -1 padding, so descriptors are only generated for real edges."""
    assert idxs_ap.dtype == I16
    assert in_ap.dtype == out_ap.dtype
    assert ap_utils.ap_is_contiguous(in_ap.ap[1:])
    assert ap_utils.ap_is_contiguous(out_ap.ap[1:])
    assert ap_utils.ap_is_contiguous(idxs_ap.ap[1:])
    assert in_ap.ap[-1][1] == out_ap.ap[-1][1] == elem_size
    assert out_ap.ap[0][1] * out_ap.ap[1][1] == (num_idxs + 127) // 128 * 128
    assert in_ap.ap[0][0] == elem_step
    dtsz = mybir.dt.size(in_ap.dtype)
    stride_bytes = elem_step * dtsz
    assert stride_bytes % 256 == 0 and stride_bytes // 256 < 256
    if num_idxs_reg is None:
        num_idxs_reg = num_idxs
    _in_ap = gpsimd.lower_ap_dma(in_ap, for_custom_bir_dma=True)
    _idxs_ap = gpsimd.lower_ap(idxs_ap)
    _out_ap = gpsimd.lower_ap(out_ap)
    return gpsimd.add_instruction(
        mybir.InstDMAGatherAnt(
            name=gpsimd.bass.get_next_instruction_name(),
            ins=[*_in_ap, _idxs_ap,
                 gpsimd.lower_val_access(gpsimd.to_reg(num_idxs_reg))],
            outs=[_out_ap],
            transpose=False, num_idxs=num_idxs, elem_size=elem_size,
            stride_bytes_256=stride_bytes // 256, gen_mode=0,
            single_packet=True, queue_num=queue_num,
            sbuf_tokens_per_rank=0, sbuf_free_dim_per_rank=0,
            sbuf_free_dim_pad_per_rank=0, sbuf_byte_offset=0,
        ))


# ----------------------------------------------------------------------------
# kernel builder
# ----------------------------------------------------------------------------
def build(cfg, meta):
    nc = bacc.Bacc("TRN2", target_bir_lowering=False, debug=False,
                   num_devices=cfg.ncores, num_swdge_queues=4,
                   dynamic_dma_scratch_size=32768)

    feats = nc.dram_tensor("feats", [cfg.shard, cfg.fin[0]], F32, kind="ExternalInput")
    aux = nc.dram_tensor("aux", [meta["aux_len"]], U8, kind="ExternalInput")
    wx, wal, brep = [], [], []
    for l in range(3):
        wx.append(nc.dram_tensor(f"wx{l}", [cfg.fin[l], cfg.geom[l]["xc"]], BF16,
                                 kind="ExternalInput"))
        wal.append(nc.dram_tensor(f"wal{l}", [cfg.fin[l], 2 * cfg.heads[l]], F32,
                                  kind="ExternalInput"))
        brep.append(nc.dram_tensor(f"brep{l}", [P, cfg.geom[l]["xc"]], F32,
                                   kind="ExternalInput"))
    ident_in = nc.dram_tensor("ident", [P, P], F32, kind="ExternalInput")
    iota_in = nc.dram_tensor("iota", [P, P], BF16, kind="ExternalInput")
    drow = nc.dram_tensor("drow", [meta["dr_len"]], BF16, kind="ExternalInput")
    iotacf_in = nc.dram_tensor("iotacf", [P, 1], F32, kind="ExternalInput")
    ones_in = nc.dram_tensor("ones", [1, P], BF16, kind="ExternalInput")
    out_sh = nc.dram_tensor("out_shard", [cfg.shard, cfg.ch[2]], F32,
                            kind="ExternalOutput")

    NQ = int(os.environ.get("GAT_NQ", "4"))
    ch_tb, ni_tb, nch_t = meta["ch_tb"], meta["ni_tb"], meta["nch_t"]
    idx_boff = meta["idx_boff"]
    off_dc, ka_t = meta["off_dc"], meta["ka_t"]
    aux_off = meta["aux_off"]
    NCH = meta["nch_max"]
    KA = int(ka_t.max())
    # sub-AllGather tile-group boundaries (pipelined collective)
    NSUB = cfg.nbanks
    bnds = cfg.bnds

    with tile.TileContext(nc) as tc:
        with (
            tc.tile_pool(name="const", bufs=1) as cp,
            tc.tile_pool(name="sb", bufs=2) as sp,
            tc.tile_pool(name="ps", bufs=1, space="PSUM") as pp,
            tc.tile_pool(name="dram", bufs=1, space="DRAM") as dp,
        ):
            # ---------------- constants ----------------
            ident = cp.tile([P, P], F32)
            nc.sync.dma_start(out=ident[:], in_=ident_in[:, :])
            iota = cp.tile([P, P], BF16)
            nc.sync.dma_start(out=iota[:], in_=iota_in[:, :])
            iotacf = cp.tile([P, 1], F32)
            nc.sync.dma_start(out=iotacf[:], in_=iotacf_in[:, :])
            ones = cp.tile([1, P], BF16)
            nc.sync.dma_start(out=ones[:], in_=ones_in[:, :])
            wx_t, wal_t, b_t, hilo_all = [], [], [], []
            for l in range(3):
                nf = cfg.fin[l] // P
                t = cp.tile([P, nf, cfg.geom[l]["xc"]], BF16, name=f"wxt{l}")
                nc.sync.dma_start(
                    out=t[:], in_=wx[l].ap().rearrange("(f p) c -> p f c", p=P))
                wx_t.append(t)
                t = cp.tile([P, nf, 2 * cfg.heads[l]], F32, name=f"walt{l}")
                nc.sync.dma_start(
                    out=t[:], in_=wal[l].ap().rearrange("(f p) c -> p f c", p=P))
                wal_t.append(t)
                t = cp.tile([P, cfg.geom[l]["xc"]], F32, name=f"bt{l}")
                nc.sync.dma_start(out=t[:], in_=brep[l].ap())
                b_t.append(t)
                hilo_all.append(cp.tile([P, cfg.tiles, 2 * cfg.heads[l]], BF16,
                                        name=f"hilo{l}"))

            # ---------------- DRAM bounces ----------------
            agin, table = [], []
            for l in range(3):
                st = cfg.geom[l]["stride"]
                agin.append(dp.tile([cfg.shard, st], U16, name=f"agin{l}"))
                tl = []
                for g in range(cfg.nbanks):
                    tl.append(dp.tile([cfg.grows[g], st], U16,
                                      name=f"table{l}_{g}", addr_space="Shared"))
                table.append(tl)

            rg = [list(range(cfg.ncores))]

            def launch_sub(l, k):
                a, b = bnds[k], bnds[k + 1]
                nc.gpsimd.collective_compute(
                    "AllGather", OP.bypass,
                    ins=[agin[l][a * P:b * P, :].opt()],
                    outs=[table[l][k][:].opt()],
                    replica_groups=rg)

            # ---------------- helpers ----------------
            def phase_a(l, t, h_tile):
                """h_tile: [128, fin] f32 SBUF -> writes agin[l] rows of tile t
                and this core's al_dst hi/lo pair into hilo_all[l]."""
                g = cfg.geom[l]
                H_ = cfg.heads[l]
                xc = g["xc"]
                nf = cfg.fin[l] // P
                hT = sp.tile([P, nf, P], F32, tag="hT")
                hTb = sp.tile([P, nf, P], BF16, tag="hTb")
                for f in range(nf):
                    tp = pp.tile([P, P], F32, space="PSUM", tag="scr", bufs=2)
                    nc.tensor.transpose(out=tp[:], in_=h_tile[:, f * P:(f + 1) * P],
                                        identity=ident[:])
                    nc.vector.tensor_copy(out=hT[:, f, :], in_=tp[:])
                    nc.scalar.activation(out=hTb[:, f, :], in_=hT[:, f, :],
                                         func=AF.Copy)
                aps = pp.tile([P, xc + 2 * H_], F32, space="PSUM",
                              tag="aps", bufs=2)
                for f in range(nf):
                    nc.tensor.matmul(out=aps[:, 0:xc], lhsT=hTb[:, f, :],
                                     rhs=wx_t[l][:, f, :],
                                     start=(f == 0), stop=(f == nf - 1))
                for f in range(nf):
                    nc.tensor.matmul(out=aps[:, xc:], lhsT=hT[:, f, :],
                                     rhs=wal_t[l][:, f, :],
                                     start=(f == 0), stop=(f == nf - 1))
                row = sp.tile([P, g["stride"]], U16, tag="row")
                rb = row[:].bitcast(BF16)
                nc.scalar.activation(out=rb[:, 0:xc], in_=aps[:, 0:xc],
                                     func=AF.Copy)
                rf = row[:].bitcast(F32)
                nc.vector.tensor_copy(out=rf[:, xc // 2:xc // 2 + H_],
                                      in_=aps[:, xc:xc + H_])
                # al_dst as bf16 hi/lo pair -> SBUF-resident hilo_all
                hi_sl = hilo_all[l][:, t, 0:H_]
                nc.scalar.activation(out=hi_sl, in_=aps[:, xc + H_:], func=AF.Copy)
                nc.vector.tensor_tensor(out=hilo_all[l][:, t, H_:2 * H_],
                                        in0=aps[:, xc + H_:], in1=hi_sl,
                                        op=OP.subtract)
                nc.scalar.dma_start(out=agin[l][t * P:(t + 1) * P, :], in_=row[:])

            def edge_loads(l, t):
                """stage 0: gt self chunk + gathers + one aux DMA."""
                g = cfg.geom[l]
                nch = int(nch_t[t])
                nchb = nch - 1

                gt_full = sp.tile([P, NCH, g["elem"]], U16, tag="g", bufs=3,
                                  name="gt")
                gt = gt_full[:, 0:nch, :]
                if t < 3:
                    # first rotation of each layer: zero the whole buffer so
                    # slots skipped by the trimmed gather hold finite bits at
                    # THIS layer's elem alignment (misaligned stale data can
                    # reinterpret as f32 NaN/huge -> exp -> Inf*0 = NaN)
                    nc.vector.memset(gt_full[:], 0)
                # chunk 0 = self-loop rows: this core's own x_aug tile
                nc.sync.dma_start(out=gt[:, 0, :],
                                  in_=agin[l][t * P:(t + 1) * P, 0:g["elem"]])
                axt = sp.tile([P, KA], U8, tag="aux", bufs=4)
                ka = int(ka_t[t])
                nc.sync.dma_start(
                    out=axt[:, 0:ka],
                    in_=aux.ap()[int(aux_off[t]):int(aux_off[t]) + P * ka]
                    .rearrange("(p m) -> p m", p=P))
                ax16 = axt[:].bitcast(I16)
                coff = 1
                for b in range(cfg.nbanks):
                    chb = int(ch_tb[t, b])
                    if chb == 0:
                        continue
                    ni = chb * P
                    rows = cfg.grows[b]
                    dma_gather_unaligned(
                        nc.gpsimd,
                        out_ap=gt[:, coff:coff + chb, :],
                        in_ap=table[l][b][0:rows, 0:g["elem"]],
                        idxs_ap=ax16[:, int(idx_boff[t, b]):
                                     int(idx_boff[t, b]) + ni // 16],
                        num_idxs=ni, elem_size=g["elem"],
                        elem_step=g["stride"], queue_num=(t + b) % NQ)
                    coff += chb
                dcol_t = axt[:].bitcast(BF16)[:, int(off_dc[t]) // 2:
                                              int(off_dc[t]) // 2 + nch]
                E = nch * P
                drow_t = sp.tile([1, NCH * P], BF16, tag="drow", bufs=3,
                                 name="drow_t")[:, 0:E]
                nc.sync.dma_start(
                    out=drow_t,
                    in_=drow.ap()[int(meta["dr_off"][t]):
                                  int(meta["dr_off"][t]) + E].unsqueeze(0))
                return dict(gt=gt, dcol_t=dcol_t, drow_t=drow_t)

            def edge_front(l, t, ld):
                """one-hot builds + al_dst expansion (deps: loads of t only)."""
                H = cfg.heads[l]
                nch = int(nch_t[t])
                E = nch * P
                dcol_t, drow_t = ld["dcol_t"], ld["drow_t"]

                oh = sp.tile([P, NCH, P], OH_DT, tag="oh", bufs=3, name="oh")[:, 0:nch, :]
                nc.vector.tensor_tensor(
                    out=oh,
                    in0=dcol_t.unsqueeze(2).to_broadcast([P, nch, P]),
                    in1=iota[:].unsqueeze(1).to_broadcast([P, nch, P]),
                    op=OP.is_equal)
                # broadcast drow across partitions via ones-matmul, then
                # ohTm[j, e] = 1 iff dst(e) == j
                dstb = sp.tile([P, NCH * P], BF16, tag="dstb", bufs=3,
                               name="dstb")[:, 0:E]
                for s0 in range(0, E, 512):
                    s1 = min(s0 + 512, E)
                    bc = pp.tile([P, 512], F32, space="PSUM", tag="scr", bufs=2,
                                 name="bc")
                    nc.tensor.matmul(out=bc[:, 0:s1 - s0], lhsT=ones[:],
                                     rhs=drow_t[:, s0:s1], start=True, stop=True)
                    nc.scalar.activation(out=dstb[:, s0:s1],
                                         in_=bc[:, 0:s1 - s0], func=AF.Copy)
                ohTm = sp.tile([P, NCH, P], OH_DT, tag="ohT", bufs=3,
                               name="ohTm")[:, 0:nch, :]
                nc.vector.tensor_scalar(
                    out=ohTm.rearrange("p c k -> p (c k)"), in0=dstb,
                    scalar1=iotacf[:, 0:1], scalar2=0.0,
                    op0=OP.subtract, op1=OP.is_equal)
                adx = pp.tile([P, NCH * H], F32, space="PSUM", tag="adx",
                              bufs=2, name="adx")[:, 0:nch * H]
                hi = hilo_all[l][:, t, 0:H]
                lo = hilo_all[l][:, t, H:2 * H]
                for c in range(nch):
                    nc.tensor.matmul(out=adx[:, c * H:(c + 1) * H],
                                     lhsT=ohTm[:, c, :],
                                     rhs=hi, start=True, stop=False)
                    nc.tensor.matmul(out=adx[:, c * H:(c + 1) * H],
                                     lhsT=ohTm[:, c, :],
                                     rhs=lo, start=False, stop=True)
                return dict(oh=oh, adx=adx)

            def edge_back(l, t, ld, fr):
                """attention weights + weighted values + segment sums."""
                g = cfg.geom[l]
                H = cfg.heads[l]
                C = cfg.ch[l]
                xc = g["xc"]
                nch = int(nch_t[t])
                gt, oh, adx = ld["gt"], fr["oh"], fr["adx"]

                gf = gt[:].bitcast(F32)
                alsrc = gf[:, :, xc // 2:xc // 2 + H]
                S = sp.tile([P, NCH, H], F32, tag="S", bufs=3, name="S")[:, 0:nch, :]
                nc.vector.tensor_tensor(
                    out=S, in0=alsrc, in1=adx.rearrange("p (c k) -> p c k", k=H),
                    op=OP.add)
                S2 = sp.tile([P, NCH, H], F32, tag="S2", bufs=3, name="S2")[:, 0:nch, :]
                nc.vector.scalar_tensor_tensor(out=S2, in0=S, scalar=0.2,
                                               in1=S, op0=OP.mult, op1=OP.max)
                gb = gt[:].bitcast(BF16)
                # chunk-halves with fully separate tiles: Scalar's exp of half
                # b overlaps Vector's multiply of half a
                na = nch // 2
                NH = (NCH + 1) // 2
                hvs = []
                for hi_, (a0, a1) in enumerate(((0, na), (na, nch))):
                    n_h = a1 - a0
                    vh = sp.tile([P, NH, xc + H], BF16, tag=f"v{hi_}",
                                 name="vh", bufs=2)[:, 0:n_h, :]
                    wx_h = sp.tile([P, NH, xc], BF16, tag=f"wexp{hi_}",
                                   name="wx_h", bufs=2)[:, 0:n_h, :]
                    nc.scalar.activation(out=vh[:, :, xc:xc + H],
                                         in_=S2[:, a0:a1, :], func=AF.Exp)
                    nc.scalar.activation(
                        out=wx_h.rearrange("p c (h x) -> p c h x", h=H),
                        in_=S2[:, a0:a1, :].unsqueeze(3)
                        .to_broadcast([P, n_h, H, C]),
                        func=AF.Exp)
                    hvs.append((a0, a1, vh, wx_h))
                for a0, a1, vh, wx_h in hvs:
                    nc.vector.tensor_tensor(out=vh[:, :, 0:xc],
                                            in0=gb[:, a0:a1, 0:xc],
                                            in1=wx_h, op=OP.mult)
                ops = pp.tile([P, xc + H], F32, space="PSUM", tag="ops", bufs=2)
                for a0, a1, vh, wx_h in hvs:
                    for c in range(a0, a1):
                        nc.tensor.matmul(out=ops[:], lhsT=oh[:, c, :],
                                         rhs=vh[:, c - a0, :],
                                         start=(c == 0), stop=(c == nch - 1))
                return ops

            def edge_epi(l, t, ops):
                """normalize + bias (+ ELU); returns h_next or writes out."""
                g = cfg.geom[l]
                H = cfg.heads[l]
                C = cfg.ch[l]
                xc = g["xc"]
                se = sp.tile([P, H], F32, tag="se", bufs=3)
                nc.vector.tensor_scalar_add(out=se[:], in0=ops[:, xc:xc + H],
                                            scalar1=1e-30)
                rs = sp.tile([P, H], F32, tag="rs", bufs=3)
                nc.vector.reciprocal(out=rs[:], in_=se[:])
                h1 = sp.tile([P, xc], F32, tag="h1", bufs=3)
                nc.vector.tensor_tensor(
                    out=h1[:].rearrange("p (h x) -> p h x", h=H),
                    in0=ops[:, 0:xc].rearrange("p (h x) -> p h x", h=H),
                    in1=rs[:].unsqueeze(2).to_broadcast([P, H, C]),
                    op=OP.mult)
                h2 = sp.tile([P, xc], F32, tag="h2", bufs=3)
                nc.vector.tensor_tensor(out=h2[:], in0=h1[:], in1=b_t[l][:],
                                        op=OP.add)
                if l == 2:
                    nc.scalar.dma_start(out=out_sh[t * P:(t + 1) * P, :], in_=h2[:])
                    return None
                m = sp.tile([P, xc], F32, tag="m", bufs=3)
                nc.vector.tensor_scalar_min(out=m[:], in0=h2[:], scalar1=0.0)
                nc.scalar.activation(out=m[:], in_=m[:], func=AF.Exp)
                hn = sp.tile([P, xc], F32, tag="hn", bufs=3)
                nc.vector.scalar_tensor_tensor(out=hn[:], in0=m[:], scalar=-1.0,
                                               in1=h2[:], op0=OP.add, op1=OP.max)
                return hn

            # ---------------- program ----------------
            sub_at = {bnds[k + 1] - 1: k for k in range(NSUB)}

            def edge_phase(l, next_l):
                lds, frs, opss = {}, {}, {}
                T = cfg.tiles
                for t in range(T + 3):
                    if t < T:
                        lds[t] = edge_loads(l, t)
                    if t - 1 >= 0 and t - 1 < T:
                        frs[t - 1] = edge_front(l, t - 1, lds[t - 1])
                    if t - 2 >= 0 and t - 2 < T:
                        u = t - 2
                        opss[u] = edge_back(l, u, lds.pop(u), frs.pop(u))
                    if t - 3 >= 0 and t - 3 < T:
                        u = t - 3
                        hn = edge_epi(l, u, opss.pop(u))
                        if next_l is not None:
                            phase_a(next_l, u, hn)
                            if u in sub_at:
                                launch_sub(next_l, sub_at[u])

            # layer 0 phase A from features (sub-collectives interleaved)
            for t in range(cfg.tiles):
                h0 = sp.tile([P, cfg.fin[0]], F32, tag="h0")
                nc.sync.dma_start(out=h0[:], in_=feats.ap()[t * P:(t + 1) * P, :])
                phase_a(0, t, h0)
                if t in sub_at:
                    launch_sub(0, sub_at[t])
            edge_phase(0, 1)
            edge_phase(1, 2)
            edge_phase(2, None)

    nc.compile()
    return nc


# ----------------------------------------------------------------------------
# entry point
# ----------------------------------------------------------------------------
def run_gat(cfg, inputs, trace=False):
    meta, aux_flats, dr_flats, gcnt_flats = preprocess(cfg, inputs["edge_index"])
    wts = make_weights(cfg, inputs)
    feats = np.asarray(inputs["features"], np.float32)
    feats_pad = np.zeros((cfg.n_pad, cfg.fin[0]), np.float32)
    feats_pad[:cfg.n_nodes] = feats

    nc = build(cfg, meta)

    shared = dict(wts)
    shared["ident"] = np.eye(P, dtype=np.float32)
    shared["iota"] = np.broadcast_to(np.arange(P, dtype=np.float32), (P, P)).astype(BF_NP)
    shared["iotacf"] = np.arange(P, dtype=np.float32).reshape(P, 1)
    shared["ones"] = np.ones((1, P), BF_NP)
    in_maps = []
    for c in range(cfg.ncores):
        m = dict(shared)
        m["feats"] = feats_pad[c * cfg.shard:(c + 1) * cfg.shard]
        m["aux"] = aux_flats[c]
        m["drow"] = dr_flats[c]
        in_maps.append(m)

    res = run_bass_kernel_spmd(nc, in_maps, core_ids=list(range(cfg.ncores)),
                               trace=trace)
    LAST_RESULT["exec_time_ns"] = res.exec_time_ns
    out = np.concatenate([res.results[c]["out_shard"] for c in range(cfg.ncores)],
                         axis=0)[:cfg.n_nodes]
    return out


def kernel(**inputs):
    cfg = Cfg()
    trace = os.environ.get("GAT_TRACE", "0") == "1"
    if trace:
        try:
            import sys as _sys, types as _types
            import trn_agent_boot.trn_boot as _tb
            _m = _types.ModuleType("antenv.axon_hooks")
            _hook = _tb._ntff_profile_via_ctypes("/opt/axon/libaxon_pjrt.so")
            _m.get_axon_ntff_profile_hook = lambda: _hook
            _m.set_axon_ntff_profile_hook = lambda h: None
            _sys.modules.setdefault("antenv.axon_hooks", _m)
            import concourse.bass_utils as _bu
            _bu.upload_artifacts = lambda tmpdir: f"file://{tmpdir}"
        except Exception:
            trace = False
    return run_gat(cfg, inputs, trace=trace).astype(np.float32)
